# revision 10
# baseline (speedup 1.0000x reference)
"""Multi-head attention Trainium2 kernel, sharded over 8 NeuronCores.

Sharding: 2 batches x 16 heads -> core c handles batch c//4, heads
[4*(c%4), 4*(c%4)+4) (tensor-parallel columns of Wq/Wk/Wv, rows of Wo).
Host pre-transposes Q/K/V per batch to [d_model, seq] so the contraction
dim lands on SBUF partitions, pre-slices weights, and sums the row-parallel
output-projection partials (plus the bv@Wo + bo term) after gathering.

Per-core dataflow (all matmuls float32r = full PE rate, ~1.5e-4 rel):
  proj:   v natural per head [tok, 64 | ones]  (ones col -> row sums)
          qT,kT [256, 2048] (head dims on partitions)
  per head:
    A: scoresT[kt] = kT_h^T q -> exp (ACT, PSUM->SBUF) -> ctxT += v_aug E^T
       row 64 of ctxT = L (free-dim layout) -> 1/L -> broadcast -> ctxT *= .
    B: scores[qt] = qT_h^T k -> exp+rowsum (ACT accum_out) -> 1/L
       per-partition -> attn = E/L -> DMA out (natural layout).
  out:    out[qt] += ctxT_h^T Wo_h  (K=64 per head), PSUM -> SBUF -> DMA.

PSUM (static, 16KB/part): 1x "big" [128,2048] + 2x "sc" [128,1024].
"""
import sys

for _p in ("/opt/trn_rl_repo", "/root/.axon_site/_ro/trn_rl_repo"):
    if _p not in sys.path:
        sys.path.append(_p)

import numpy as np
import concourse.bass as bass
import concourse.tile as tile
from concourse import bacc, mybir
from concourse.bass_utils import run_bass_kernel_spmd

dt = mybir.dt
F32 = dt.float32
F32R = dt.float32r
AF = mybir.ActivationFunctionType

D_MODEL = 1024
N_HEADS = 16
D_K = 64                 # head dim
SEQ = 2048
B = 2
N_CORES = 8
HEADS_PER_CORE = 4
D_CORE = HEADS_PER_CORE * D_K  # 256 projection cols per core
SCALE = 1.0 / np.sqrt(D_K)

KT = SEQ // 128          # 16 token chunks of 128
DC = D_MODEL // 128      # 8 d_model chunks


def build_nc():
    nc = bacc.Bacc("TRN2", target_bir_lowering=False, debug=False, num_devices=N_CORES)

    QTd = nc.dram_tensor("qt_in", [D_MODEL, SEQ], F32R, kind="ExternalInput")
    KTd = nc.dram_tensor("kt_in", [D_MODEL, SEQ], F32R, kind="ExternalInput")
    VTd = nc.dram_tensor("vt_in", [D_MODEL, SEQ], F32R, kind="ExternalInput")
    Wqd = nc.dram_tensor("wq_in", [D_MODEL, D_CORE], F32R, kind="ExternalInput")
    Wkd = nc.dram_tensor("wk_in", [D_MODEL, D_CORE], F32R, kind="ExternalInput")
    Wvd = nc.dram_tensor("wv_in", [D_MODEL, D_CORE], F32R, kind="ExternalInput")
    Wod = nc.dram_tensor("wo_in", [D_CORE, D_MODEL], F32R, kind="ExternalInput")
    bqd = nc.dram_tensor("bq_in", [D_CORE], F32, kind="ExternalInput")
    bkd = nc.dram_tensor("bk_in", [D_CORE], F32, kind="ExternalInput")

    attn_out = nc.dram_tensor("attn_out", [HEADS_PER_CORE, SEQ, SEQ], F32,
                              kind="ExternalOutput")
    out_part = nc.dram_tensor("out_part", [SEQ, D_MODEL], F32,
                              kind="ExternalOutput")
    import os
    dbg = None
    if os.environ.get("KDEBUG"):
        dbg = {
            "ctx": nc.dram_tensor("ctx_dbg", [HEADS_PER_CORE, D_K, SEQ], F32,
                                  kind="ExternalOutput"),
            "ctxu": nc.dram_tensor("ctxu_dbg", [HEADS_PER_CORE, D_K, SEQ], F32,
                                   kind="ExternalOutput"),
            "l": nc.dram_tensor("l_dbg", [HEADS_PER_CORE, 1, SEQ], F32,
                                kind="ExternalOutput"),
            "r": nc.dram_tensor("r_dbg", [HEADS_PER_CORE, 1, SEQ], F32,
                                kind="ExternalOutput"),
            "vaug": nc.dram_tensor("vaug_dbg", [HEADS_PER_CORE, 128, KT, D_K + 1],
                                   F32, kind="ExternalOutput"),
        }

    with tile.TileContext(nc) as tc:
        build_kernel(tc, QTd, KTd, VTd, Wqd, Wkd, Wvd, Wod, bqd, bkd,
                     attn_out, out_part, dbg)
    nc.compile()
    return nc


def build_kernel(tc, QTd, KTd, VTd, Wqd, Wkd, Wvd, Wod, bqd, bkd,
                 attn_out, out_part, dbg=None):
    nc = tc.nc
    with (
        tc.tile_pool(name="weights", bufs=1) as wpool,
        tc.tile_pool(name="proj", bufs=1) as projpool,
        tc.tile_pool(name="work", bufs=2) as workpool,
        tc.tile_pool(name="lpool", bufs=1) as lpool,
        tc.tile_pool(name="small", bufs=4) as small,
        tc.tile_pool(name="psbig", bufs=1, space="PSUM") as psbig,
        tc.tile_pool(name="pssc", bufs=2, space="PSUM") as pssc,
    ):
        # ---- weight + bias loads ----
        wq_sb = wpool.tile([128, DC, D_CORE], F32R, name="wq_sb")
        wk_sb = wpool.tile([128, DC, D_CORE], F32R, name="wk_sb")
        wv_sb = wpool.tile([128, DC, D_CORE], F32R, name="wv_sb")
        nc.sync.dma_start(wq_sb[:], Wqd.ap().rearrange("(c p) n -> p c n", p=128))
        nc.sync.dma_start(wk_sb[:], Wkd.ap().rearrange("(c p) n -> p c n", p=128))
        nc.sync.dma_start(wv_sb[:], Wvd.ap().rearrange("(c p) n -> p c n", p=128))
        wo_sb = []
        for h in range(HEADS_PER_CORE):
            t = wpool.tile([D_K, D_MODEL], F32R, name=f"wo_sb{h}")
            nc.sync.dma_start(t[:], Wod[h * D_K:(h + 1) * D_K, :])
            wo_sb.append(t)
        bq_sb = wpool.tile([128, 2], F32, name="bq_sb")
        bk_sb = wpool.tile([128, 2], F32, name="bk_sb")
        nc.sync.dma_start(bq_sb[:], bqd.ap().rearrange("(c p) -> p c", p=128))
        nc.sync.dma_start(bk_sb[:], bkd.ap().rearrange("(c p) -> p c", p=128))
        ones_sb = wpool.tile([1, D_K], F32, name="ones_sb")
        nc.vector.memset(ones_sb[:], 1.0)

        def stream_tile(name):
            return workpool.tile([128, SEQ], F32R, tag="et", name=name)

        # ---- V projection: v natural [tok, 64]+ones col per head ----
        # (memset can't target f32r; copy from an f32 ones tile instead)
        ones_col = wpool.tile([128, KT], F32, name="ones_col")
        nc.vector.memset(ones_col[:], 1.0)
        v_aug = []
        for h in range(HEADS_PER_CORE):
            t = projpool.tile([128, KT, D_K + 1], F32R, name=f"v_aug{h}")
            nc.vector.tensor_copy(
                t[:, :, D_K:D_K + 1],
                ones_col[:].rearrange("p (k o) -> p k o", o=1))
            v_aug.append(t)

        vps8 = psbig.tile([128, 8, D_CORE], F32, tag="big", name="vps8")
        vps4 = [pssc.tile([128, 4, D_CORE], F32, tag="sc", name=f"vps4_{i}")
                for i in range(2)]

        def vtarget(t):
            if t < 8:
                return vps8[:, t, :]
            return vps4[(t - 8) // 4][:, (t - 8) % 4, :]

        # Two 1KB slices share each 2KB PSUM bank and start=True clears the
        # whole bank: emit start only on the bank's first write (even t) and
        # stop only on its last (odd t).
        for dc in range(DC):
            vt_t = stream_tile(f"vt{dc}")
            nc.sync.dma_start(vt_t[:], VTd[dc * 128:(dc + 1) * 128, :])
            for t in range(KT):
                nc.tensor.matmul(
                    vtarget(t), vt_t[:, t * 128:(t + 1) * 128],
                    wv_sb[:, dc, :],
                    start=(dc == 0 and t % 2 == 0),
                    stop=(dc == DC - 1 and t % 2 == 1))
        for t in range(KT):
            for h in range(HEADS_PER_CORE):
                nc.vector.tensor_copy(
                    v_aug[h][:, t, 0:D_K], vtarget(t)[:, h * D_K:(h + 1) * D_K])

        # ---- Q/K projections -> qT, kT [128, 2, 2048] (dout on partitions) ----
        qT_sb = projpool.tile([128, 2, SEQ], F32R, name="qT_sb")
        kT_sb = projpool.tile([128, 2, SEQ], F32R, name="kT_sb")
        for (Xd, W, bias, dst) in ((QTd, wq_sb, bq_sb, qT_sb),
                                   (KTd, wk_sb, bk_sb, kT_sb)):
            ps0 = psbig.tile([128, SEQ], F32, tag="big", name=f"pp0{Xd.name}")
            ps1 = [pssc.tile([128, 1024], F32, tag="sc", name=f"pp1{Xd.name}{i}")
                   for i in range(2)]
            for dc in range(DC):
                x_t = stream_tile(f"x{Xd.name}{dc}")
                nc.sync.dma_start(x_t[:], Xd[dc * 128:(dc + 1) * 128, :])
                for n in range(4):
                    nc.tensor.matmul(
                        ps0[:, n * 512:(n + 1) * 512],
                        W[:, dc, 0:128],
                        x_t[:, n * 512:(n + 1) * 512],
                        start=(dc == 0), stop=(dc == DC - 1))
                for n in range(4):
                    nc.tensor.matmul(
                        ps1[n // 2][:, (n % 2) * 512:(n % 2 + 1) * 512],
                        W[:, dc, 128:256],
                        x_t[:, n * 512:(n + 1) * 512],
                        start=(dc == 0), stop=(dc == DC - 1))
            nc.vector.tensor_scalar_add(dst[:, 0, :], ps0[:], bias[:, 0:1])
            for i in range(2):
                nc.vector.tensor_scalar_add(
                    dst[:, 1, i * 1024:(i + 1) * 1024], ps1[i][:], bias[:, 1:2])

        def head_ap(t, h, sl):
            p0 = 64 * (h % 2)
            return t[p0:p0 + D_K, h // 2, sl]

        # ---- attention per head ----
        ctxT = []
        for h in range(HEADS_PER_CORE):
            # Phase A: scoresT chunks -> exp -> ctxT accumulation
            ctxps = psbig.tile([128, SEQ], F32, tag="big", name=f"ctxps{h}")
            for kt in range(KT):
                et = workpool.tile([128, SEQ], F32R, tag="et", name=f"et{h}_{kt}")
                for half in range(2):
                    ps = pssc.tile([128, 1024], F32, tag="sc",
                                   name=f"sA{h}_{kt}_{half}")
                    for i in range(2):
                        sl = slice(half * 1024 + i * 512, half * 1024 + (i + 1) * 512)
                        nc.tensor.matmul(
                            ps[:, i * 512:(i + 1) * 512],
                            head_ap(kT_sb, h, slice(kt * 128, (kt + 1) * 128)),
                            head_ap(qT_sb, h, sl),
                            start=True, stop=True)
                    nc.scalar.activation(
                        et[:, half * 1024:(half + 1) * 1024], ps[:],
                        AF.Exp, scale=float(SCALE))
                for n in range(SEQ // 512):
                    nc.tensor.matmul(
                        ctxps[0:D_K + 1, n * 512:(n + 1) * 512],
                        v_aug[h][:, kt, :],
                        et[:, n * 512:(n + 1) * 512],
                        start=(kt == 0), stop=(kt == KT - 1))

            # drain ctx + normalize by row-sums (free-dim layout).
            # L sits on PSUM partition 64; DVE can't shift partitions, so
            # copy it partition-aligned to SBUF, then DMA it to partition 0.
            ctx_h = projpool.tile([D_K, SEQ], F32R, name=f"ctx{h}")
            nc.vector.tensor_copy(ctx_h[:], ctxps[0:D_K, :])
            ltmp = workpool.tile([D_K + 1, SEQ], F32, tag="et", name=f"ltmp{h}")
            nc.vector.tensor_copy(ltmp[D_K:D_K + 1, :], ctxps[D_K:D_K + 1, :])
            lrow = lpool.tile([1, SEQ], F32, tag="lrow", name=f"lrow{h}")
            nc.sync.dma_start(lrow[:], ltmp[D_K:D_K + 1, :])
            recipf = lpool.tile([1, SEQ], F32, tag="recipf", name=f"recipf{h}")
            nc.vector.reciprocal(recipf[:], lrow[:])
            rb = [pssc.tile([128, 1024], F32, tag="sc", name=f"rb{h}_{i}")
                  for i in range(2)]
            for n in range(SEQ // 512):
                nc.tensor.matmul(rb[n // 2][0:D_K, (n % 2) * 512:(n % 2 + 1) * 512],
                                 ones_sb[:], recipf[:, n * 512:(n + 1) * 512],
                                 start=True, stop=True)
            if dbg is not None:
                nc.sync.dma_start(dbg["ctxu"][h], ctx_h[:].bitcast(F32))
                nc.sync.dma_start(dbg["l"][h], lrow[:])
                nc.sync.dma_start(dbg["r"][h], recipf[:])
                nc.sync.dma_start(dbg["vaug"][h], v_aug[h][:].bitcast(F32))
            for i in range(2):
                nc.vector.tensor_mul(ctx_h[:, i * 1024:(i + 1) * 1024],
                                     ctx_h[:, i * 1024:(i + 1) * 1024],
                                     rb[i][0:D_K, :])
            if dbg is not None:
                nc.sync.dma_start(dbg["ctx"][h], ctx_h[:].bitcast(F32))
            ctxT.append(ctx_h)

            # Phase B: natural scores -> exp(+rowsum) -> normalize -> DMA out
            for qt in range(KT):
                en = workpool.tile([128, SEQ], F32, tag="en", name=f"en{h}_{qt}")
                lh = small.tile([128, 2], F32, tag="lh", name=f"lh{h}_{qt}")
                for half in range(2):
                    ps = pssc.tile([128, 1024], F32, tag="sc",
                                   name=f"sB{h}_{qt}_{half}")
                    for i in range(2):
                        sl = slice(half * 1024 + i * 512, half * 1024 + (i + 1) * 512)
                        nc.tensor.matmul(
                            ps[:, i * 512:(i + 1) * 512],
                            head_ap(qT_sb, h, slice(qt * 128, (qt + 1) * 128)),
                            head_ap(kT_sb, h, sl),
                            start=True, stop=True)
                    nc.scalar.activation(
                        en[:, half * 1024:(half + 1) * 1024], ps[:],
                        AF.Exp, scale=float(SCALE),
                        accum_out=lh[:, half:half + 1])
                lsum = small.tile([128, 1], F32, tag="lsum", name=f"ls{h}_{qt}")
                nc.vector.tensor_add(lsum[:], lh[:, 0:1], lh[:, 1:2])
                rc = small.tile([128, 1], F32, tag="rc", name=f"rc{h}_{qt}")
                nc.vector.reciprocal(rc[:], lsum[:])
                nc.vector.tensor_scalar_mul(en[:], en[:], rc[:])
                nc.sync.dma_start(
                    attn_out[h, qt * 128:(qt + 1) * 128, :], en[:])

        # ---- output projection (partial over this core's 256 dims) ----
        for qt in range(KT):
            pso = pssc.tile([128, 1024], F32, tag="sc", name=f"po{qt}")
            for h in range(HEADS_PER_CORE):
                for n in range(2):
                    nc.tensor.matmul(
                        pso[:, n * 512:(n + 1) * 512],
                        ctxT[h][:, qt * 128:(qt + 1) * 128],
                        wo_sb[h][:, n * 512:(n + 1) * 512],
                        start=(h == 0), stop=(h == HEADS_PER_CORE - 1))
            outs = workpool.tile([128, D_MODEL], F32, tag="outs", name=f"os{qt}")
            nc.vector.tensor_copy(outs[:], pso[:])
            nc.sync.dma_start(out_part[qt * 128:(qt + 1) * 128, :], outs[:])


_NC_CACHE = None


def get_nc():
    global _NC_CACHE
    if _NC_CACHE is None:
        _NC_CACHE = build_nc()
    return _NC_CACHE


def make_in_maps(Q, K, V, Wq, bq, Wk, bk, Wv, bv, Wo, bo):
    in_maps = []
    qkvT = {}
    for b in range(B):
        qkvT[b] = (np.ascontiguousarray(Q[b].T), np.ascontiguousarray(K[b].T),
                   np.ascontiguousarray(V[b].T))
    for c in range(N_CORES):
        b, g = divmod(c, 4)
        cs = slice(g * D_CORE, (g + 1) * D_CORE)
        qt, kt, vt = qkvT[b]
        in_maps.append({
            "qt_in": qt, "kt_in": kt, "vt_in": vt,
            "wq_in": np.ascontiguousarray(Wq[:, cs]),
            "wk_in": np.ascontiguousarray(Wk[:, cs]),
            "wv_in": np.ascontiguousarray(Wv[:, cs]),
            "wo_in": np.ascontiguousarray(Wo[cs, :]),
            "bq_in": np.ascontiguousarray(bq[cs]),
            "bk_in": np.ascontiguousarray(bk[cs]),
        })
    return in_maps


def assemble(results, bv, Wo, bo):
    attn = np.empty((B, N_HEADS, SEQ, SEQ), dtype=np.float32)
    out = np.zeros((B, SEQ, D_MODEL), dtype=np.float32)
    const = (bv.astype(np.float64) @ Wo.astype(np.float64) + bo).astype(np.float32)
    for c in range(N_CORES):
        b, g = divmod(c, 4)
        attn[b, g * HEADS_PER_CORE:(g + 1) * HEADS_PER_CORE] = results[c]["attn_out"]
        out[b] += results[c]["out_part"]
    out += const
    return out, attn


def kernel(Q, K, V, Wq, bq, Wk, bk, Wv, bv, Wo, bo, _trace=False, **_ignored):
    nc = get_nc()
    in_maps = make_in_maps(Q, K, V, Wq, bq, Wk, bk, Wv, bv, Wo, bo)
    res = run_bass_kernel_spmd(nc, in_maps, list(range(N_CORES)), trace=_trace)
    out, attn = assemble(res.results, bv, Wo, bo)
    kernel.last_results = res
    return out, attn


if __name__ == "__main__":
    rng = np.random.default_rng(0)
    ins = {
        "Q": rng.standard_normal((B, SEQ, D_MODEL), dtype=np.float32),
        "K": rng.standard_normal((B, SEQ, D_MODEL), dtype=np.float32),
        "V": rng.standard_normal((B, SEQ, D_MODEL), dtype=np.float32),
    }
    s = 1.0 / np.sqrt(D_MODEL)
    for name in ("q", "k", "v", "o"):
        ins[f"W{name}"] = rng.standard_normal((D_MODEL, D_MODEL), dtype=np.float32) * s
        ins[f"b{name}"] = rng.standard_normal((D_MODEL,), dtype=np.float32) * s
    out, attn = kernel(**ins)
    print("ran", out.shape, attn.shape)


# revision 11
# speedup vs baseline: 1.0874x; 1.0874x over previous
"""Multi-head attention Trainium2 kernel, sharded over 8 NeuronCores.

Sharding: 2 batches x 16 heads -> core c handles batch c//4, heads
[4*(c%4), 4*(c%4)+4) (tensor-parallel columns of Wq/Wk/Wv, rows of Wo).
Host pre-transposes Q/K/V per batch to [d_model, seq] so the contraction
dim lands on SBUF partitions, pre-slices weights, and sums the row-parallel
output-projection partials (plus the bv@Wo + bo term) after gathering.

Per-core dataflow (all matmuls float32r = full PE rate, ~1.5e-4 rel):
  proj:   v natural per head [tok, 64 | ones]  (ones col -> row sums)
          qT,kT [256, 2048] (head dims on partitions)
  per head:
    A: scoresT[kt] = kT_h^T q -> exp (ACT, PSUM->SBUF) -> ctxT += v_aug E^T
       row 64 of ctxT = L (free-dim layout) -> 1/L -> broadcast -> ctxT *= .
    B: scores[qt] = qT_h^T k -> exp+rowsum (ACT accum_out) -> 1/L
       per-partition -> attn = E/L -> DMA out (natural layout).
  out:    out[qt] += ctxT_h^T Wo_h  (K=64 per head), PSUM -> SBUF -> DMA.

PSUM (static, 16KB/part): 1x "big" [128,2048] + 2x "sc" [128,1024].
"""
import sys

for _p in ("/opt/trn_rl_repo", "/root/.axon_site/_ro/trn_rl_repo"):
    if _p not in sys.path:
        sys.path.append(_p)

import numpy as np
import concourse.bass as bass
import concourse.tile as tile
from concourse import bacc, mybir
from concourse.bass_utils import run_bass_kernel_spmd

dt = mybir.dt
F32 = dt.float32
F32R = dt.float32r
B16 = dt.bfloat16
# dtype for the attention-side matmuls (scores, AV, out-proj).  bf16 runs
# the PE at 1 col/cycle; float32r measured ~3x slower (fp32_mode=HIGH).
import os as _os
DT_ATT = F32R if _os.environ.get("KF32R") else B16
AF = mybir.ActivationFunctionType

D_MODEL = 1024
N_HEADS = 16
D_K = 64                 # head dim
SEQ = 2048
B = 2
N_CORES = 8
HEADS_PER_CORE = 4
D_CORE = HEADS_PER_CORE * D_K  # 256 projection cols per core
SCALE = 1.0 / np.sqrt(D_K)

KT = SEQ // 128          # 16 token chunks of 128
DC = D_MODEL // 128      # 8 d_model chunks


def build_nc():
    nc = bacc.Bacc("TRN2", target_bir_lowering=False, debug=False, num_devices=N_CORES)

    QTd = nc.dram_tensor("qt_in", [D_MODEL, SEQ], F32R, kind="ExternalInput")
    KTd = nc.dram_tensor("kt_in", [D_MODEL, SEQ], F32R, kind="ExternalInput")
    VTd = nc.dram_tensor("vt_in", [D_MODEL, SEQ], F32R, kind="ExternalInput")
    Wqd = nc.dram_tensor("wq_in", [D_MODEL, D_CORE], F32R, kind="ExternalInput")
    Wkd = nc.dram_tensor("wk_in", [D_MODEL, D_CORE], F32R, kind="ExternalInput")
    Wvd = nc.dram_tensor("wv_in", [D_MODEL, D_CORE], F32R, kind="ExternalInput")
    Wod = nc.dram_tensor("wo_in", [D_CORE, D_MODEL], F32R, kind="ExternalInput")
    bqd = nc.dram_tensor("bq_in", [D_CORE], F32, kind="ExternalInput")
    bkd = nc.dram_tensor("bk_in", [D_CORE], F32, kind="ExternalInput")

    attn_out = nc.dram_tensor("attn_out", [HEADS_PER_CORE, SEQ, SEQ], F32,
                              kind="ExternalOutput")
    out_part = nc.dram_tensor("out_part", [SEQ, D_MODEL], F32,
                              kind="ExternalOutput")
    import os
    dbg = None
    if os.environ.get("KDEBUG"):
        dbg = {
            "ctx": nc.dram_tensor("ctx_dbg", [HEADS_PER_CORE, D_K, SEQ], F32,
                                  kind="ExternalOutput"),
            "ctxu": nc.dram_tensor("ctxu_dbg", [HEADS_PER_CORE, D_K, SEQ], F32,
                                   kind="ExternalOutput"),
            "l": nc.dram_tensor("l_dbg", [HEADS_PER_CORE, 1, SEQ], F32,
                                kind="ExternalOutput"),
            "r": nc.dram_tensor("r_dbg", [HEADS_PER_CORE, 1, SEQ], F32,
                                kind="ExternalOutput"),
            "vaug": nc.dram_tensor("vaug_dbg", [HEADS_PER_CORE, 128, KT, D_K + 1],
                                   F32, kind="ExternalOutput"),
        }

    with tile.TileContext(nc) as tc:
        build_kernel(tc, QTd, KTd, VTd, Wqd, Wkd, Wvd, Wod, bqd, bkd,
                     attn_out, out_part, dbg)
    nc.compile()
    return nc


def build_kernel(tc, QTd, KTd, VTd, Wqd, Wkd, Wvd, Wod, bqd, bkd,
                 attn_out, out_part, dbg=None):
    nc = tc.nc
    with (
        tc.tile_pool(name="weights", bufs=1) as wpool,
        tc.tile_pool(name="proj", bufs=1) as projpool,
        tc.tile_pool(name="work", bufs=2) as workpool,
        tc.tile_pool(name="lpool", bufs=1) as lpool,
        tc.tile_pool(name="small", bufs=4) as small,
        tc.tile_pool(name="psbig", bufs=1, space="PSUM") as psbig,
        tc.tile_pool(name="pssc", bufs=2, space="PSUM") as pssc,
    ):
        # ---- weight + bias loads ----
        wq_sb = wpool.tile([128, DC, D_CORE], F32R, name="wq_sb")
        wk_sb = wpool.tile([128, DC, D_CORE], F32R, name="wk_sb")
        wv_sb = wpool.tile([128, DC, D_CORE], F32R, name="wv_sb")
        nc.sync.dma_start(wq_sb[:], Wqd.ap().rearrange("(c p) n -> p c n", p=128))
        nc.sync.dma_start(wk_sb[:], Wkd.ap().rearrange("(c p) n -> p c n", p=128))
        nc.sync.dma_start(wv_sb[:], Wvd.ap().rearrange("(c p) n -> p c n", p=128))
        wo_sb = []
        for h in range(HEADS_PER_CORE):
            t = wpool.tile([D_K, D_MODEL], F32R, name=f"wo_sb{h}")
            nc.sync.dma_start(t[:], Wod[h * D_K:(h + 1) * D_K, :])
            if DT_ATT is not F32R:
                tb = wpool.tile([D_K, D_MODEL], DT_ATT, name=f"wo_b{h}")
                nc.vector.tensor_copy(tb[:], t[:].bitcast(F32))
                t = tb
            wo_sb.append(t)
        bq_sb = wpool.tile([128, 2], F32, name="bq_sb")
        bk_sb = wpool.tile([128, 2], F32, name="bk_sb")
        nc.sync.dma_start(bq_sb[:], bqd.ap().rearrange("(c p) -> p c", p=128))
        nc.sync.dma_start(bk_sb[:], bkd.ap().rearrange("(c p) -> p c", p=128))
        ones_sb = wpool.tile([1, D_K], F32, name="ones_sb")
        nc.vector.memset(ones_sb[:], 1.0)

        def stream_tile(name):
            return workpool.tile([128, SEQ], F32R, tag="et", name=name)

        # ---- V projection: v natural [tok, 64]+ones col per head ----
        # (memset can't target f32r; copy from an f32 ones tile instead)
        ones_col = wpool.tile([128, KT], F32, name="ones_col")
        nc.vector.memset(ones_col[:], 1.0)
        v_aug = []
        for h in range(HEADS_PER_CORE):
            t = projpool.tile([128, KT, D_K + 1], DT_ATT, name=f"v_aug{h}")
            nc.vector.tensor_copy(
                t[:, :, D_K:D_K + 1],
                ones_col[:].rearrange("p (k o) -> p k o", o=1))
            v_aug.append(t)

        vps8 = psbig.tile([128, 8, D_CORE], F32, tag="big", name="vps8")
        vps4 = [pssc.tile([128, 4, D_CORE], F32, tag="sc", name=f"vps4_{i}")
                for i in range(2)]

        def vtarget(t):
            if t < 8:
                return vps8[:, t, :]
            return vps4[(t - 8) // 4][:, (t - 8) % 4, :]

        # Two 1KB slices share each 2KB PSUM bank and start=True clears the
        # whole bank: emit start only on the bank's first write (even t) and
        # stop only on its last (odd t).
        for dc in range(DC):
            vt_t = stream_tile(f"vt{dc}")
            nc.sync.dma_start(vt_t[:], VTd[dc * 128:(dc + 1) * 128, :])
            for t in range(KT):
                nc.tensor.matmul(
                    vtarget(t), vt_t[:, t * 128:(t + 1) * 128],
                    wv_sb[:, dc, :],
                    start=(dc == 0 and t % 2 == 0),
                    stop=(dc == DC - 1 and t % 2 == 1))
        for t in range(KT):
            for h in range(HEADS_PER_CORE):
                nc.vector.tensor_copy(
                    v_aug[h][:, t, 0:D_K], vtarget(t)[:, h * D_K:(h + 1) * D_K])

        # ---- Q/K projections -> qT, kT [128, 2, 2048] (dout on partitions) ----
        qT_sb = projpool.tile([128, 2, SEQ], DT_ATT, name="qT_sb")
        kT_sb = projpool.tile([128, 2, SEQ], DT_ATT, name="kT_sb")
        for (Xd, W, bias, dst) in ((QTd, wq_sb, bq_sb, qT_sb),
                                   (KTd, wk_sb, bk_sb, kT_sb)):
            ps0 = psbig.tile([128, SEQ], F32, tag="big", name=f"pp0{Xd.name}")
            ps1 = [pssc.tile([128, 1024], F32, tag="sc", name=f"pp1{Xd.name}{i}")
                   for i in range(2)]
            for dc in range(DC):
                x_t = stream_tile(f"x{Xd.name}{dc}")
                nc.sync.dma_start(x_t[:], Xd[dc * 128:(dc + 1) * 128, :])
                for n in range(4):
                    nc.tensor.matmul(
                        ps0[:, n * 512:(n + 1) * 512],
                        W[:, dc, 0:128],
                        x_t[:, n * 512:(n + 1) * 512],
                        start=(dc == 0), stop=(dc == DC - 1))
                for n in range(4):
                    nc.tensor.matmul(
                        ps1[n // 2][:, (n % 2) * 512:(n % 2 + 1) * 512],
                        W[:, dc, 128:256],
                        x_t[:, n * 512:(n + 1) * 512],
                        start=(dc == 0), stop=(dc == DC - 1))
            nc.vector.tensor_scalar_add(dst[:, 0, :], ps0[:], bias[:, 0:1])
            for i in range(2):
                nc.vector.tensor_scalar_add(
                    dst[:, 1, i * 1024:(i + 1) * 1024], ps1[i][:], bias[:, 1:2])

        def head_ap(t, h, sl):
            p0 = 64 * (h % 2)
            return t[p0:p0 + D_K, h // 2, sl]

        # ---- attention per head ----
        ctxT = []
        for h in range(HEADS_PER_CORE):
            # Phase A: scoresT chunks -> exp -> ctxT accumulation
            ctxps = psbig.tile([128, SEQ], F32, tag="big", name=f"ctxps{h}")
            for kt in range(KT):
                et = workpool.tile([128, SEQ], DT_ATT, tag="et", name=f"et{h}_{kt}")
                for half in range(2):
                    ps = pssc.tile([128, 1024], F32, tag="sc",
                                   name=f"sA{h}_{kt}_{half}")
                    for i in range(2):
                        sl = slice(half * 1024 + i * 512, half * 1024 + (i + 1) * 512)
                        nc.tensor.matmul(
                            ps[:, i * 512:(i + 1) * 512],
                            head_ap(kT_sb, h, slice(kt * 128, (kt + 1) * 128)),
                            head_ap(qT_sb, h, sl),
                            start=True, stop=True)
                    nc.scalar.activation(
                        et[:, half * 1024:(half + 1) * 1024], ps[:],
                        AF.Exp, scale=float(SCALE))
                for n in range(SEQ // 512):
                    nc.tensor.matmul(
                        ctxps[0:D_K + 1, n * 512:(n + 1) * 512],
                        v_aug[h][:, kt, :],
                        et[:, n * 512:(n + 1) * 512],
                        start=(kt == 0), stop=(kt == KT - 1))

            # drain ctx + normalize by row-sums (free-dim layout).
            # L sits on PSUM partition 64; DVE can't shift partitions, so
            # copy it partition-aligned to SBUF, then DMA it to partition 0.
            ctx_h = projpool.tile([D_K, SEQ], DT_ATT, name=f"ctx{h}")
            nc.vector.tensor_copy(ctx_h[:], ctxps[0:D_K, :])
            ltmp = workpool.tile([D_K + 1, SEQ], F32, tag="et", name=f"ltmp{h}")
            nc.vector.tensor_copy(ltmp[D_K:D_K + 1, :], ctxps[D_K:D_K + 1, :])
            lrow = lpool.tile([1, SEQ], F32, tag="lrow", name=f"lrow{h}")
            nc.sync.dma_start(lrow[:], ltmp[D_K:D_K + 1, :])
            recipf = lpool.tile([1, SEQ], F32, tag="recipf", name=f"recipf{h}")
            nc.vector.reciprocal(recipf[:], lrow[:])
            rb = [pssc.tile([128, 1024], F32, tag="sc", name=f"rb{h}_{i}")
                  for i in range(2)]
            for n in range(SEQ // 512):
                nc.tensor.matmul(rb[n // 2][0:D_K, (n % 2) * 512:(n % 2 + 1) * 512],
                                 ones_sb[:], recipf[:, n * 512:(n + 1) * 512],
                                 start=True, stop=True)
            if dbg is not None:
                nc.sync.dma_start(dbg["ctxu"][h], ctx_h[:].bitcast(F32))
                nc.sync.dma_start(dbg["l"][h], lrow[:])
                nc.sync.dma_start(dbg["r"][h], recipf[:])
                nc.sync.dma_start(dbg["vaug"][h], v_aug[h][:].bitcast(F32))
            for i in range(2):
                nc.vector.tensor_mul(ctx_h[:, i * 1024:(i + 1) * 1024],
                                     ctx_h[:, i * 1024:(i + 1) * 1024],
                                     rb[i][0:D_K, :])
            if dbg is not None:
                nc.sync.dma_start(dbg["ctx"][h], ctx_h[:].bitcast(F32))
            ctxT.append(ctx_h)

            # Phase B: natural scores -> exp(+rowsum) -> normalize -> DMA out
            for qt in range(KT):
                en = workpool.tile([128, SEQ], F32, tag="en", name=f"en{h}_{qt}")
                lh = small.tile([128, 2], F32, tag="lh", name=f"lh{h}_{qt}")
                for half in range(2):
                    ps = pssc.tile([128, 1024], F32, tag="sc",
                                   name=f"sB{h}_{qt}_{half}")
                    for i in range(2):
                        sl = slice(half * 1024 + i * 512, half * 1024 + (i + 1) * 512)
                        nc.tensor.matmul(
                            ps[:, i * 512:(i + 1) * 512],
                            head_ap(qT_sb, h, slice(qt * 128, (qt + 1) * 128)),
                            head_ap(kT_sb, h, sl),
                            start=True, stop=True)
                    nc.scalar.activation(
                        en[:, half * 1024:(half + 1) * 1024], ps[:],
                        AF.Exp, scale=float(SCALE),
                        accum_out=lh[:, half:half + 1])
                lsum = small.tile([128, 1], F32, tag="lsum", name=f"ls{h}_{qt}")
                nc.vector.tensor_add(lsum[:], lh[:, 0:1], lh[:, 1:2])
                rc = small.tile([128, 1], F32, tag="rc", name=f"rc{h}_{qt}")
                nc.vector.reciprocal(rc[:], lsum[:])
                nc.vector.tensor_scalar_mul(en[:], en[:], rc[:])
                nc.sync.dma_start(
                    attn_out[h, qt * 128:(qt + 1) * 128, :], en[:])

        # ---- output projection (partial over this core's 256 dims) ----
        for qt in range(KT):
            pso = pssc.tile([128, 1024], F32, tag="sc", name=f"po{qt}")
            for h in range(HEADS_PER_CORE):
                for n in range(2):
                    nc.tensor.matmul(
                        pso[:, n * 512:(n + 1) * 512],
                        ctxT[h][:, qt * 128:(qt + 1) * 128],
                        wo_sb[h][:, n * 512:(n + 1) * 512],
                        start=(h == 0), stop=(h == HEADS_PER_CORE - 1))
            outs = workpool.tile([128, D_MODEL], F32, tag="outs", name=f"os{qt}")
            nc.vector.tensor_copy(outs[:], pso[:])
            nc.sync.dma_start(out_part[qt * 128:(qt + 1) * 128, :], outs[:])


_NC_CACHE = None


def get_nc():
    global _NC_CACHE
    if _NC_CACHE is None:
        _NC_CACHE = build_nc()
    return _NC_CACHE


def make_in_maps(Q, K, V, Wq, bq, Wk, bk, Wv, bv, Wo, bo):
    in_maps = []
    qkvT = {}
    for b in range(B):
        qkvT[b] = (np.ascontiguousarray(Q[b].T), np.ascontiguousarray(K[b].T),
                   np.ascontiguousarray(V[b].T))
    for c in range(N_CORES):
        b, g = divmod(c, 4)
        cs = slice(g * D_CORE, (g + 1) * D_CORE)
        qt, kt, vt = qkvT[b]
        in_maps.append({
            "qt_in": qt, "kt_in": kt, "vt_in": vt,
            "wq_in": np.ascontiguousarray(Wq[:, cs]),
            "wk_in": np.ascontiguousarray(Wk[:, cs]),
            "wv_in": np.ascontiguousarray(Wv[:, cs]),
            "wo_in": np.ascontiguousarray(Wo[cs, :]),
            "bq_in": np.ascontiguousarray(bq[cs]),
            "bk_in": np.ascontiguousarray(bk[cs]),
        })
    return in_maps


def assemble(results, bv, Wo, bo):
    attn = np.empty((B, N_HEADS, SEQ, SEQ), dtype=np.float32)
    out = np.zeros((B, SEQ, D_MODEL), dtype=np.float32)
    const = (bv.astype(np.float64) @ Wo.astype(np.float64) + bo).astype(np.float32)
    for c in range(N_CORES):
        b, g = divmod(c, 4)
        attn[b, g * HEADS_PER_CORE:(g + 1) * HEADS_PER_CORE] = results[c]["attn_out"]
        out[b] += results[c]["out_part"]
    out += const
    return out, attn


def kernel(Q, K, V, Wq, bq, Wk, bk, Wv, bv, Wo, bo, _trace=False, **_ignored):
    nc = get_nc()
    in_maps = make_in_maps(Q, K, V, Wq, bq, Wk, bk, Wv, bv, Wo, bo)
    res = run_bass_kernel_spmd(nc, in_maps, list(range(N_CORES)), trace=_trace)
    out, attn = assemble(res.results, bv, Wo, bo)
    kernel.last_results = res
    return out, attn


if __name__ == "__main__":
    rng = np.random.default_rng(0)
    ins = {
        "Q": rng.standard_normal((B, SEQ, D_MODEL), dtype=np.float32),
        "K": rng.standard_normal((B, SEQ, D_MODEL), dtype=np.float32),
        "V": rng.standard_normal((B, SEQ, D_MODEL), dtype=np.float32),
    }
    s = 1.0 / np.sqrt(D_MODEL)
    for name in ("q", "k", "v", "o"):
        ins[f"W{name}"] = rng.standard_normal((D_MODEL, D_MODEL), dtype=np.float32) * s
        ins[f"b{name}"] = rng.standard_normal((D_MODEL,), dtype=np.float32) * s
    out, attn = kernel(**ins)
    print("ran", out.shape, attn.shape)


# revision 15
# speedup vs baseline: 1.3086x; 1.2034x over previous
"""Multi-head attention Trainium2 kernel, sharded over 8 NeuronCores.

Sharding: 2 batches x 16 heads -> core c handles batch c//4, heads
[4*(c%4), 4*(c%4)+4) (tensor-parallel columns of Wq/Wk/Wv, rows of Wo).
Host pre-transposes Q/K/V per batch to [d_model, seq] so the contraction
dim lands on SBUF partitions, pre-slices weights, and sums the row-parallel
output-projection partials (plus the bv@Wo + bo term) after gathering.

Per-core dataflow (all matmuls float32r = full PE rate, ~1.5e-4 rel):
  proj:   v natural per head [tok, 64 | ones]  (ones col -> row sums)
          qT,kT [256, 2048] (head dims on partitions)
  per head:
    A: scoresT[kt] = kT_h^T q -> exp (ACT, PSUM->SBUF) -> ctxT += v_aug E^T
       row 64 of ctxT = L (free-dim layout) -> 1/L -> broadcast -> ctxT *= .
    B: scores[qt] = qT_h^T k -> exp+rowsum (ACT accum_out) -> 1/L
       per-partition -> attn = E/L -> DMA out (natural layout).
  out:    out[qt] += ctxT_h^T Wo_h  (K=64 per head), PSUM -> SBUF -> DMA.

PSUM (static, 16KB/part): 1x "big" [128,2048] + 2x "sc" [128,1024].
"""
import sys

for _p in ("/opt/trn_rl_repo", "/root/.axon_site/_ro/trn_rl_repo"):
    if _p not in sys.path:
        sys.path.append(_p)

import numpy as np
import concourse.bass as bass
import concourse.tile as tile
from concourse import bacc, mybir
from concourse.bass_utils import run_bass_kernel_spmd

dt = mybir.dt
F32 = dt.float32
F32R = dt.float32r
B16 = dt.bfloat16
# dtype for the attention-side matmuls (scores, AV, out-proj).  bf16 runs
# the PE at 1 col/cycle; float32r measured ~3x slower (fp32_mode=HIGH).
import os as _os
DT_ATT = F32R if _os.environ.get("KF32R") else B16
AF = mybir.ActivationFunctionType

D_MODEL = 1024
N_HEADS = 16
D_K = 64                 # head dim
SEQ = 2048
B = 2
N_CORES = 8
HEADS_PER_CORE = 4
D_CORE = HEADS_PER_CORE * D_K  # 256 projection cols per core
SCALE = 1.0 / np.sqrt(D_K)

KT = SEQ // 128          # 16 token chunks of 128
DC = D_MODEL // 128      # 8 d_model chunks


def build_nc():
    nc = bacc.Bacc("TRN2", target_bir_lowering=False, debug=False, num_devices=N_CORES)

    QTd = nc.dram_tensor("qt_in", [D_MODEL, SEQ], F32R, kind="ExternalInput")
    KTd = nc.dram_tensor("kt_in", [D_MODEL, SEQ], F32R, kind="ExternalInput")
    VTd = nc.dram_tensor("vt_in", [D_MODEL, SEQ], F32R, kind="ExternalInput")
    Wqd = nc.dram_tensor("wq_in", [D_MODEL, D_CORE], F32R, kind="ExternalInput")
    Wkd = nc.dram_tensor("wk_in", [D_MODEL, D_CORE], F32R, kind="ExternalInput")
    Wvd = nc.dram_tensor("wv_in", [D_MODEL, D_CORE], F32R, kind="ExternalInput")
    Wod = nc.dram_tensor("wo_in", [D_CORE, D_MODEL], F32R, kind="ExternalInput")
    bqd = nc.dram_tensor("bq_in", [D_CORE], F32, kind="ExternalInput")
    idd = nc.dram_tensor("id_in", [16, 16], F32, kind="ExternalInput")
    bkd = nc.dram_tensor("bk_in", [D_CORE], F32, kind="ExternalInput")

    attn_out = nc.dram_tensor("attn_out", [HEADS_PER_CORE, SEQ, SEQ], F32,
                              kind="ExternalOutput")
    out_part = nc.dram_tensor("out_part", [SEQ, D_MODEL], F32,
                              kind="ExternalOutput")
    import os
    dbg = None
    if os.environ.get("KDEBUG"):
        dbg = {
            "ctx": nc.dram_tensor("ctx_dbg", [HEADS_PER_CORE, D_K, SEQ], F32,
                                  kind="ExternalOutput"),
            "ctxu": nc.dram_tensor("ctxu_dbg", [HEADS_PER_CORE, D_K, SEQ], F32,
                                   kind="ExternalOutput"),
            "l": nc.dram_tensor("l_dbg", [HEADS_PER_CORE, 1, SEQ], F32,
                                kind="ExternalOutput"),
            "r": nc.dram_tensor("r_dbg", [HEADS_PER_CORE, 1, SEQ], F32,
                                kind="ExternalOutput"),
            "vaug": nc.dram_tensor("vaug_dbg", [HEADS_PER_CORE, 128, KT, D_K + 1],
                                   F32, kind="ExternalOutput"),
        }

    with tile.TileContext(nc) as tc:
        build_kernel(tc, QTd, KTd, VTd, Wqd, Wkd, Wvd, Wod, bqd, bkd, idd,
                     attn_out, out_part, dbg)
    nc.compile()
    return nc


def build_kernel(tc, QTd, KTd, VTd, Wqd, Wkd, Wvd, Wod, bqd, bkd, idd,
                 attn_out, out_part, dbg=None):
    nc = tc.nc
    with (
        tc.tile_pool(name="weights", bufs=1) as wpool,
        tc.tile_pool(name="proj", bufs=1) as projpool,
        tc.tile_pool(name="work", bufs=2) as workpool,
        tc.tile_pool(name="lpool", bufs=1) as lpool,
        tc.tile_pool(name="small", bufs=4) as small,
        tc.tile_pool(name="psbig", bufs=1, space="PSUM") as psbig,
        tc.tile_pool(name="pssc", bufs=2, space="PSUM") as pssc,
    ):
        # ---- weight + bias loads ----
        wq_sb = wpool.tile([128, DC, D_CORE], F32R, name="wq_sb")
        wk_sb = wpool.tile([128, DC, D_CORE], F32R, name="wk_sb")
        wv_sb = wpool.tile([128, DC, D_CORE], F32R, name="wv_sb")
        nc.sync.dma_start(wq_sb[:], Wqd.ap().rearrange("(c p) n -> p c n", p=128))
        nc.sync.dma_start(wk_sb[:], Wkd.ap().rearrange("(c p) n -> p c n", p=128))
        nc.sync.dma_start(wv_sb[:], Wvd.ap().rearrange("(c p) n -> p c n", p=128))
        wo_sb = []
        for h in range(HEADS_PER_CORE):
            t = wpool.tile([D_K, D_MODEL], F32R, name=f"wo_sb{h}")
            nc.sync.dma_start(t[:], Wod[h * D_K:(h + 1) * D_K, :])
            if DT_ATT is not F32R:
                tb = wpool.tile([D_K, D_MODEL], DT_ATT, name=f"wo_b{h}")
                nc.vector.tensor_copy(tb[:], t[:].bitcast(F32))
                t = tb
            wo_sb.append(t)
        bq_sb = wpool.tile([128, 2], F32, name="bq_sb")
        bk_sb = wpool.tile([128, 2], F32, name="bk_sb")
        nc.sync.dma_start(bq_sb[:], bqd.ap().rearrange("(c p) -> p c", p=128))
        nc.sync.dma_start(bk_sb[:], bkd.ap().rearrange("(c p) -> p c", p=128))
        ones_big = wpool.tile([128, D_K], F32, name="ones_big")
        nc.vector.memset(ones_big[:], 1.0)
        id16 = wpool.tile([16, 16], F32, name="id16")
        nc.sync.dma_start(id16[:], idd[:])

        def stream_tile(name):
            return workpool.tile([128, SEQ], F32R, tag="et", name=name)

        # ---- V projection: v natural [tok, 64]+ones col per head ----
        # (memset can't target f32r; copy from an f32 ones tile instead)
        ones_col = wpool.tile([128, KT], F32, name="ones_col")
        nc.vector.memset(ones_col[:], 1.0)
        v_aug = []
        for h in range(HEADS_PER_CORE):
            t = projpool.tile([128, KT, D_K + 1], DT_ATT, name=f"v_aug{h}")
            nc.vector.tensor_copy(
                t[:, :, D_K:D_K + 1],
                ones_col[:].rearrange("p (k o) -> p k o", o=1))
            v_aug.append(t)

        vps8 = psbig.tile([128, 8, D_CORE], F32, tag="big", name="vps8")
        vps4 = [pssc.tile([128, 4, D_CORE], F32, tag="sc", name=f"vps4_{i}")
                for i in range(2)]

        def vtarget(t):
            if t < 8:
                return vps8[:, t, :]
            return vps4[(t - 8) // 4][:, (t - 8) % 4, :]

        # Two 1KB slices share each 2KB PSUM bank and start=True clears the
        # whole bank: emit start only on the bank's first write (even t) and
        # stop only on its last (odd t).
        for dc in range(DC):
            vt_t = stream_tile(f"vt{dc}")
            nc.sync.dma_start(vt_t[:], VTd[dc * 128:(dc + 1) * 128, :])
            for t in range(KT):
                nc.tensor.matmul(
                    vtarget(t), vt_t[:, t * 128:(t + 1) * 128],
                    wv_sb[:, dc, :],
                    start=(dc == 0 and t % 2 == 0),
                    stop=(dc == DC - 1 and t % 2 == 1))
        for t in range(KT):
            for h in range(HEADS_PER_CORE):
                nc.vector.tensor_copy(
                    v_aug[h][:, t, 0:D_K], vtarget(t)[:, h * D_K:(h + 1) * D_K])

        # ---- Q/K projections -> qT, kT [128, 2, 2048] (dout on partitions) ----
        qT_sb = projpool.tile([128, 2, SEQ], DT_ATT, name="qT_sb")
        kT_sb = projpool.tile([128, 2, SEQ], DT_ATT, name="kT_sb")
        for (Xd, W, bias, dst) in ((QTd, wq_sb, bq_sb, qT_sb),
                                   (KTd, wk_sb, bk_sb, kT_sb)):
            ps0 = psbig.tile([128, SEQ], F32, tag="big", name=f"pp0{Xd.name}")
            ps1 = [pssc.tile([128, 1024], F32, tag="sc", name=f"pp1{Xd.name}{i}")
                   for i in range(2)]
            for dc in range(DC):
                x_t = stream_tile(f"x{Xd.name}{dc}")
                nc.sync.dma_start(x_t[:], Xd[dc * 128:(dc + 1) * 128, :])
                for n in range(4):
                    nc.tensor.matmul(
                        ps0[:, n * 512:(n + 1) * 512],
                        W[:, dc, 0:128],
                        x_t[:, n * 512:(n + 1) * 512],
                        start=(dc == 0), stop=(dc == DC - 1))
                for n in range(4):
                    nc.tensor.matmul(
                        ps1[n // 2][:, (n % 2) * 512:(n % 2 + 1) * 512],
                        W[:, dc, 128:256],
                        x_t[:, n * 512:(n + 1) * 512],
                        start=(dc == 0), stop=(dc == DC - 1))
            nc.vector.tensor_scalar_add(dst[:, 0, :], ps0[:], bias[:, 0:1])
            for i in range(2):
                nc.vector.tensor_scalar_add(
                    dst[:, 1, i * 1024:(i + 1) * 1024], ps1[i][:], bias[:, 1:2])

        def head_ap(t, h, sl):
            p0 = 64 * (h % 2)
            return t[p0:p0 + D_K, h // 2, sl]

        # ---- attention: pipeline B(h) with A(h+1) to keep PE dense ----
        ctxT = []
        lse_neg = [None] * HEADS_PER_CORE
        rt16s = [None] * HEADS_PER_CORE

        def phase_a_step(h, kt, ctxps):
            """scoresT chunk kt -> exp -> AV accumulation (+ones row -> L)."""
            et = workpool.tile([128, SEQ], DT_ATT, tag="et", name=f"et{h}_{kt}")
            for half in range(2):
                ps = pssc.tile([128, 1024], F32, tag="sc",
                               name=f"sA{h}_{kt}_{half}")
                for i in range(2):
                    sl = slice(half * 1024 + i * 512, half * 1024 + (i + 1) * 512)
                    nc.tensor.matmul(
                        ps[:, i * 512:(i + 1) * 512],
                        head_ap(kT_sb, h, slice(kt * 128, (kt + 1) * 128)),
                        head_ap(qT_sb, h, sl),
                        start=True, stop=True)
                nc.scalar.activation(
                    et[:, half * 1024:(half + 1) * 1024], ps[:],
                    AF.Exp, scale=float(SCALE))
            for n in range(SEQ // 512):
                nc.tensor.matmul(
                    ctxps[0:D_K + 1, n * 512:(n + 1) * 512],
                    v_aug[h][:, kt, :],
                    et[:, n * 512:(n + 1) * 512],
                    start=(kt == 0), stop=(kt == KT - 1))

        def finish_a(h, ctxps):
            """Drain ctx, move L (psum row 64, free layout) into per-partition
            -ln(L) for phase B's exp bias, and 1/L rows for ctx normalize."""
            ctx_h = projpool.tile([D_K, SEQ], DT_ATT, name=f"ctx{h}")
            nc.vector.tensor_copy(ctx_h[:], ctxps[0:D_K, :])
            ltmp = workpool.tile([D_K + 1, SEQ], F32, tag="et", name=f"ltmp{h}")
            nc.vector.tensor_copy(ltmp[D_K:D_K + 1, :], ctxps[D_K:D_K + 1, :])
            l16 = lpool.tile([16, 128], F32, tag="l16", name=f"l16_{h}")
            nc.sync.dma_start(l16[:], ltmp[D_K:D_K + 1, :])
            ln16 = lpool.tile([16, 128], F32, tag="ln16", name=f"ln16_{h}")
            nc.scalar.activation(ln16[:], l16[:], AF.Ln)
            lnp = pssc.tile([128, 1024], F32, tag="sc", name=f"lnp{h}")
            nc.tensor.transpose(lnp[0:128, 0:16], ln16[:], id16[:])
            ln_pp = small.tile([128, 16], F32, tag="lnpp", name=f"lnpp{h}")
            nc.vector.tensor_scalar_mul(ln_pp[:], lnp[0:128, 0:16], -1.0)
            lse_neg[h] = ln_pp
            rt = lpool.tile([16, 128], F32, tag="rt16", name=f"rt16_{h}")
            nc.vector.reciprocal(rt[:], l16[:])
            rt16s[h] = rt
            ctxT.append(ctx_h)
            return ctx_h

        def norm_ctx(h):
            """ctx_h *= 1/L broadcast along partitions (ones-outer per chunk)."""
            ctx_h = ctxT[h]
            rt = rt16s[h]
            rfree = lpool.tile([1, SEQ], F32, tag="rfree", name=f"rfree{h}")
            nc.sync.dma_start(rfree[:], rt[:])
            rb = [pssc.tile([128, 1024], F32, tag="sc", name=f"rb{h}_{i}")
                  for i in range(2)]
            for n in range(SEQ // 512):
                nc.tensor.matmul(
                    rb[n // 2][0:D_K, (n % 2) * 512:(n % 2 + 1) * 512],
                    ones_big[0:1, :], rfree[:, n * 512:(n + 1) * 512],
                    start=True, stop=True)
            for i in range(2):
                nc.vector.tensor_mul(ctx_h[:, i * 1024:(i + 1) * 1024],
                                     ctx_h[:, i * 1024:(i + 1) * 1024],
                                     rb[i][0:D_K, :])

        def phase_b_step(h, qt):
            """Natural scores chunk -> exp(x/8 - lnL) = normalized attn -> DMA."""
            en = workpool.tile([128, SEQ], F32, tag="en", name=f"en{h}_{qt}")
            for half in range(2):
                ps = pssc.tile([128, 1024], F32, tag="sc",
                               name=f"sB{h}_{qt}_{half}")
                for i in range(2):
                    sl = slice(half * 1024 + i * 512, half * 1024 + (i + 1) * 512)
                    nc.tensor.matmul(
                        ps[:, i * 512:(i + 1) * 512],
                        head_ap(qT_sb, h, slice(qt * 128, (qt + 1) * 128)),
                        head_ap(kT_sb, h, sl),
                        start=True, stop=True)
                nc.scalar.activation(
                    en[:, half * 1024:(half + 1) * 1024], ps[:],
                    AF.Exp, scale=float(SCALE),
                    bias=lse_neg[h][:, qt:qt + 1])
            nc.sync.dma_start(attn_out[h, qt * 128:(qt + 1) * 128, :], en[:])

        ctxps = psbig.tile([128, SEQ], F32, tag="big", name="ctxps0")
        for kt in range(KT):
            phase_a_step(0, kt, ctxps)
        finish_a(0, ctxps)
        for h in range(HEADS_PER_CORE):
            if h + 1 < HEADS_PER_CORE:
                nxt = psbig.tile([128, SEQ], F32, tag="big", name=f"ctxps{h+1}")
                for i in range(KT):
                    phase_a_step(h + 1, i, nxt)
                    phase_b_step(h, i)
                finish_a(h + 1, nxt)
            else:
                for i in range(KT):
                    phase_b_step(h, i)
            norm_ctx(h)

        # ---- output projection (partial over this core's 256 dims) ----
        for qt in range(KT):
            pso = pssc.tile([128, 1024], F32, tag="sc", name=f"po{qt}")
            for h in range(HEADS_PER_CORE):
                for n in range(2):
                    nc.tensor.matmul(
                        pso[:, n * 512:(n + 1) * 512],
                        ctxT[h][:, qt * 128:(qt + 1) * 128],
                        wo_sb[h][:, n * 512:(n + 1) * 512],
                        start=(h == 0), stop=(h == HEADS_PER_CORE - 1))
            outs = workpool.tile([128, D_MODEL], F32, tag="outs", name=f"os{qt}")
            nc.vector.tensor_copy(outs[:], pso[:])
            nc.sync.dma_start(out_part[qt * 128:(qt + 1) * 128, :], outs[:])


_NC_CACHE = None


def get_nc():
    global _NC_CACHE
    if _NC_CACHE is None:
        _NC_CACHE = build_nc()
    return _NC_CACHE


def make_in_maps(Q, K, V, Wq, bq, Wk, bk, Wv, bv, Wo, bo):
    in_maps = []
    qkvT = {}
    for b in range(B):
        qkvT[b] = (np.ascontiguousarray(Q[b].T), np.ascontiguousarray(K[b].T),
                   np.ascontiguousarray(V[b].T))
    for c in range(N_CORES):
        b, g = divmod(c, 4)
        cs = slice(g * D_CORE, (g + 1) * D_CORE)
        qt, kt, vt = qkvT[b]
        in_maps.append({
            "qt_in": qt, "kt_in": kt, "vt_in": vt,
            "wq_in": np.ascontiguousarray(Wq[:, cs]),
            "wk_in": np.ascontiguousarray(Wk[:, cs]),
            "wv_in": np.ascontiguousarray(Wv[:, cs]),
            "wo_in": np.ascontiguousarray(Wo[cs, :]),
            "bq_in": np.ascontiguousarray(bq[cs]),
            "bk_in": np.ascontiguousarray(bk[cs]),
            "id_in": np.eye(16, dtype=np.float32),
        })
    return in_maps


def assemble(results, bv, Wo, bo):
    attn = np.empty((B, N_HEADS, SEQ, SEQ), dtype=np.float32)
    out = np.zeros((B, SEQ, D_MODEL), dtype=np.float32)
    const = (bv.astype(np.float64) @ Wo.astype(np.float64) + bo).astype(np.float32)
    for c in range(N_CORES):
        b, g = divmod(c, 4)
        attn[b, g * HEADS_PER_CORE:(g + 1) * HEADS_PER_CORE] = results[c]["attn_out"]
        out[b] += results[c]["out_part"]
    out += const
    return out, attn


def kernel(Q, K, V, Wq, bq, Wk, bk, Wv, bv, Wo, bo, _trace=False, **_ignored):
    nc = get_nc()
    in_maps = make_in_maps(Q, K, V, Wq, bq, Wk, bk, Wv, bv, Wo, bo)
    res = run_bass_kernel_spmd(nc, in_maps, list(range(N_CORES)), trace=_trace)
    out, attn = assemble(res.results, bv, Wo, bo)
    kernel.last_results = res
    return out, attn


if __name__ == "__main__":
    rng = np.random.default_rng(0)
    ins = {
        "Q": rng.standard_normal((B, SEQ, D_MODEL), dtype=np.float32),
        "K": rng.standard_normal((B, SEQ, D_MODEL), dtype=np.float32),
        "V": rng.standard_normal((B, SEQ, D_MODEL), dtype=np.float32),
    }
    s = 1.0 / np.sqrt(D_MODEL)
    for name in ("q", "k", "v", "o"):
        ins[f"W{name}"] = rng.standard_normal((D_MODEL, D_MODEL), dtype=np.float32) * s
        ins[f"b{name}"] = rng.standard_normal((D_MODEL,), dtype=np.float32) * s
    out, attn = kernel(**ins)
    print("ran", out.shape, attn.shape)


# revision 16
# speedup vs baseline: 1.3415x; 1.0251x over previous
"""Multi-head attention Trainium2 kernel, sharded over 8 NeuronCores.

Sharding: 2 batches x 16 heads -> core c handles batch c//4, heads
[4*(c%4), 4*(c%4)+4) (tensor-parallel columns of Wq/Wk/Wv, rows of Wo).
Host pre-transposes Q/K/V per batch to [d_model, seq] so the contraction
dim lands on SBUF partitions, pre-slices weights, and sums the row-parallel
output-projection partials (plus the bv@Wo + bo term) after gathering.

Per-core dataflow (all matmuls float32r = full PE rate, ~1.5e-4 rel):
  proj:   v natural per head [tok, 64 | ones]  (ones col -> row sums)
          qT,kT [256, 2048] (head dims on partitions)
  per head:
    A: scoresT[kt] = kT_h^T q -> exp (ACT, PSUM->SBUF) -> ctxT += v_aug E^T
       row 64 of ctxT = L (free-dim layout) -> 1/L -> broadcast -> ctxT *= .
    B: scores[qt] = qT_h^T k -> exp+rowsum (ACT accum_out) -> 1/L
       per-partition -> attn = E/L -> DMA out (natural layout).
  out:    out[qt] += ctxT_h^T Wo_h  (K=64 per head), PSUM -> SBUF -> DMA.

PSUM (static, 16KB/part): 1x "big" [128,2048] + 2x "sc" [128,1024].
"""
import sys

for _p in ("/opt/trn_rl_repo", "/root/.axon_site/_ro/trn_rl_repo"):
    if _p not in sys.path:
        sys.path.append(_p)

import numpy as np
import concourse.bass as bass
import concourse.tile as tile
from concourse import bacc, mybir
from concourse.bass_utils import run_bass_kernel_spmd

dt = mybir.dt
F32 = dt.float32
F32R = dt.float32r
B16 = dt.bfloat16
# dtype for the attention-side matmuls (scores, AV, out-proj).  bf16 runs
# the PE at 1 col/cycle; float32r measured ~3x slower (fp32_mode=HIGH).
import os as _os
DT_ATT = F32R if _os.environ.get("KF32R") else B16
AF = mybir.ActivationFunctionType

D_MODEL = 1024
N_HEADS = 16
D_K = 64                 # head dim
SEQ = 2048
B = 2
N_CORES = 8
HEADS_PER_CORE = 4
D_CORE = HEADS_PER_CORE * D_K  # 256 projection cols per core
SCALE = 1.0 / np.sqrt(D_K)

KT = SEQ // 128          # 16 token chunks of 128
DC = D_MODEL // 128      # 8 d_model chunks


def build_nc():
    nc = bacc.Bacc("TRN2", target_bir_lowering=False, debug=False, num_devices=N_CORES)

    QTd = nc.dram_tensor("qt_in", [D_MODEL, SEQ], F32R, kind="ExternalInput")
    KTd = nc.dram_tensor("kt_in", [D_MODEL, SEQ], F32R, kind="ExternalInput")
    VTd = nc.dram_tensor("vt_in", [D_MODEL, SEQ], F32R, kind="ExternalInput")
    Wqd = nc.dram_tensor("wq_in", [D_MODEL, D_CORE], F32R, kind="ExternalInput")
    Wkd = nc.dram_tensor("wk_in", [D_MODEL, D_CORE], F32R, kind="ExternalInput")
    Wvd = nc.dram_tensor("wv_in", [D_MODEL, D_CORE], F32R, kind="ExternalInput")
    Wod = nc.dram_tensor("wo_in", [D_CORE, D_MODEL], F32R, kind="ExternalInput")
    bqd = nc.dram_tensor("bq_in", [D_CORE], F32, kind="ExternalInput")
    idd = nc.dram_tensor("id_in", [16, 16], F32, kind="ExternalInput")
    bkd = nc.dram_tensor("bk_in", [D_CORE], F32, kind="ExternalInput")

    attn_out = nc.dram_tensor("attn_out", [HEADS_PER_CORE, SEQ, SEQ], F32,
                              kind="ExternalOutput")
    out_part = nc.dram_tensor("out_part", [SEQ, D_MODEL], F32,
                              kind="ExternalOutput")
    import os
    dbg = None
    if os.environ.get("KDEBUG"):
        dbg = {
            "ctx": nc.dram_tensor("ctx_dbg", [HEADS_PER_CORE, D_K, SEQ], F32,
                                  kind="ExternalOutput"),
            "ctxu": nc.dram_tensor("ctxu_dbg", [HEADS_PER_CORE, D_K, SEQ], F32,
                                   kind="ExternalOutput"),
            "l": nc.dram_tensor("l_dbg", [HEADS_PER_CORE, 1, SEQ], F32,
                                kind="ExternalOutput"),
            "r": nc.dram_tensor("r_dbg", [HEADS_PER_CORE, 1, SEQ], F32,
                                kind="ExternalOutput"),
            "vaug": nc.dram_tensor("vaug_dbg", [HEADS_PER_CORE, 128, KT, D_K + 1],
                                   F32, kind="ExternalOutput"),
        }

    with tile.TileContext(nc) as tc:
        build_kernel(tc, QTd, KTd, VTd, Wqd, Wkd, Wvd, Wod, bqd, bkd, idd,
                     attn_out, out_part, dbg)
    nc.compile()
    return nc


def build_kernel(tc, QTd, KTd, VTd, Wqd, Wkd, Wvd, Wod, bqd, bkd, idd,
                 attn_out, out_part, dbg=None):
    nc = tc.nc
    with (
        tc.tile_pool(name="weights", bufs=1) as wpool,
        tc.tile_pool(name="proj", bufs=1) as projpool,
        tc.tile_pool(name="work", bufs=2) as workpool,
        tc.tile_pool(name="lpool", bufs=1) as lpool,
        tc.tile_pool(name="small", bufs=4) as small,
        tc.tile_pool(name="psbig", bufs=1, space="PSUM") as psbig,
        tc.tile_pool(name="pssc", bufs=2, space="PSUM") as pssc,
    ):
        # ---- weight + bias loads ----
        wq_sb = wpool.tile([128, DC, D_CORE], F32R, name="wq_sb")
        wk_sb = wpool.tile([128, DC, D_CORE], F32R, name="wk_sb")
        wv_sb = wpool.tile([128, DC, D_CORE], F32R, name="wv_sb")
        nc.sync.dma_start(wq_sb[:], Wqd.ap().rearrange("(c p) n -> p c n", p=128))
        nc.sync.dma_start(wk_sb[:], Wkd.ap().rearrange("(c p) n -> p c n", p=128))
        nc.sync.dma_start(wv_sb[:], Wvd.ap().rearrange("(c p) n -> p c n", p=128))
        wo_sb = []
        for h in range(HEADS_PER_CORE):
            t = wpool.tile([D_K, D_MODEL], F32R, name=f"wo_sb{h}")
            nc.sync.dma_start(t[:], Wod[h * D_K:(h + 1) * D_K, :])
            if DT_ATT is not F32R:
                tb = wpool.tile([D_K, D_MODEL], DT_ATT, name=f"wo_b{h}")
                nc.vector.tensor_copy(tb[:], t[:].bitcast(F32))
                t = tb
            wo_sb.append(t)
        bq_sb = wpool.tile([128, 2], F32, name="bq_sb")
        bk_sb = wpool.tile([128, 2], F32, name="bk_sb")
        nc.sync.dma_start(bq_sb[:], bqd.ap().rearrange("(c p) -> p c", p=128))
        nc.sync.dma_start(bk_sb[:], bkd.ap().rearrange("(c p) -> p c", p=128))
        ones_big = wpool.tile([128, D_K], F32, name="ones_big")
        nc.vector.memset(ones_big[:], 1.0)
        id16 = wpool.tile([16, 16], F32, name="id16")
        nc.sync.dma_start(id16[:], idd[:])

        def stream_tile(name):
            return workpool.tile([128, SEQ], F32R, tag="et", name=name)

        # ---- V projection: v natural [tok, 64]+ones col per head ----
        # (memset can't target f32r; copy from an f32 ones tile instead)
        ones_col = wpool.tile([128, KT], F32, name="ones_col")
        nc.vector.memset(ones_col[:], 1.0)
        v_aug = []
        for h in range(HEADS_PER_CORE):
            t = projpool.tile([128, KT, D_K + 1], DT_ATT, name=f"v_aug{h}")
            nc.vector.tensor_copy(
                t[:, :, D_K:D_K + 1],
                ones_col[:].rearrange("p (k o) -> p k o", o=1))
            v_aug.append(t)

        vps8 = psbig.tile([128, 8, D_CORE], F32, tag="big", name="vps8")
        vps4 = [pssc.tile([128, 4, D_CORE], F32, tag="sc", name=f"vps4_{i}")
                for i in range(2)]

        def vtarget(t):
            if t < 8:
                return vps8[:, t, :]
            return vps4[(t - 8) // 4][:, (t - 8) % 4, :]

        # Two 1KB slices share each 2KB PSUM bank and start=True clears the
        # whole bank: emit start only on the bank's first write (even t) and
        # stop only on its last (odd t).
        for dc in range(DC):
            vt_t = stream_tile(f"vt{dc}")
            nc.sync.dma_start(vt_t[:], VTd[dc * 128:(dc + 1) * 128, :])
            for t in range(KT):
                nc.tensor.matmul(
                    vtarget(t), vt_t[:, t * 128:(t + 1) * 128],
                    wv_sb[:, dc, :],
                    start=(dc == 0 and t % 2 == 0),
                    stop=(dc == DC - 1 and t % 2 == 1))
        for t in range(KT):
            for h in range(HEADS_PER_CORE):
                nc.vector.tensor_copy(
                    v_aug[h][:, t, 0:D_K], vtarget(t)[:, h * D_K:(h + 1) * D_K])

        # ---- Q/K projections -> qT, kT [128, 2, 2048] (dout on partitions) ----
        qT_sb = projpool.tile([128, 2, SEQ], DT_ATT, name="qT_sb")
        kT_sb = projpool.tile([128, 2, SEQ], DT_ATT, name="kT_sb")
        for (Xd, W, bias, dst) in ((QTd, wq_sb, bq_sb, qT_sb),
                                   (KTd, wk_sb, bk_sb, kT_sb)):
            ps0 = psbig.tile([128, SEQ], F32, tag="big", name=f"pp0{Xd.name}")
            ps1 = [pssc.tile([128, 1024], F32, tag="sc", name=f"pp1{Xd.name}{i}")
                   for i in range(2)]
            for dc in range(DC):
                x_t = stream_tile(f"x{Xd.name}{dc}")
                nc.sync.dma_start(x_t[:], Xd[dc * 128:(dc + 1) * 128, :])
                for n in range(4):
                    nc.tensor.matmul(
                        ps0[:, n * 512:(n + 1) * 512],
                        W[:, dc, 0:128],
                        x_t[:, n * 512:(n + 1) * 512],
                        start=(dc == 0), stop=(dc == DC - 1))
                for n in range(4):
                    nc.tensor.matmul(
                        ps1[n // 2][:, (n % 2) * 512:(n % 2 + 1) * 512],
                        W[:, dc, 128:256],
                        x_t[:, n * 512:(n + 1) * 512],
                        start=(dc == 0), stop=(dc == DC - 1))
            nc.vector.tensor_scalar_add(dst[:, 0, :], ps0[:], bias[:, 0:1])
            for i in range(2):
                nc.vector.tensor_scalar_add(
                    dst[:, 1, i * 1024:(i + 1) * 1024], ps1[i][:], bias[:, 1:2])

        def head_ap(t, h, sl):
            p0 = 64 * (h % 2)
            return t[p0:p0 + D_K, h // 2, sl]

        # ---- attention: pipeline B(h) with A(h+1) to keep PE dense ----
        ctxT = []
        lse_neg = [None] * HEADS_PER_CORE
        rt16s = [None] * HEADS_PER_CORE

        def a_scores(h, kt):
            """scoresT chunk kt -> exp -> E^T tile (bf16)."""
            et = workpool.tile([128, SEQ], DT_ATT, tag="et", name=f"et{h}_{kt}")
            for half in range(2):
                ps = pssc.tile([128, 1024], F32, tag="sc",
                               name=f"sA{h}_{kt}_{half}")
                for i in range(2):
                    sl = slice(half * 1024 + i * 512, half * 1024 + (i + 1) * 512)
                    nc.tensor.matmul(
                        ps[:, i * 512:(i + 1) * 512],
                        head_ap(kT_sb, h, slice(kt * 128, (kt + 1) * 128)),
                        head_ap(qT_sb, h, sl),
                        start=True, stop=True)
                nc.scalar.activation(
                    et[:, half * 1024:(half + 1) * 1024], ps[:],
                    AF.Exp, scale=float(SCALE))
            return et

        def a_av(h, kt, et, ctxps):
            for n in range(SEQ // 512):
                nc.tensor.matmul(
                    ctxps[0:D_K + 1, n * 512:(n + 1) * 512],
                    v_aug[h][:, kt, :],
                    et[:, n * 512:(n + 1) * 512],
                    start=(kt == 0), stop=(kt == KT - 1))

        def finish_a(h, ctxps):
            """Drain ctx, move L (psum row 64, free layout) into per-partition
            -ln(L) for phase B's exp bias, and 1/L rows for ctx normalize."""
            ctx_h = projpool.tile([D_K, SEQ], DT_ATT, name=f"ctx{h}")
            nc.vector.tensor_copy(ctx_h[:], ctxps[0:D_K, :])
            ltmp = workpool.tile([D_K + 1, SEQ], F32, tag="et", name=f"ltmp{h}")
            nc.vector.tensor_copy(ltmp[D_K:D_K + 1, :], ctxps[D_K:D_K + 1, :])
            l16 = lpool.tile([16, 128], F32, tag="l16", name=f"l16_{h}")
            nc.sync.dma_start(l16[:], ltmp[D_K:D_K + 1, :])
            ln16 = lpool.tile([16, 128], F32, tag="ln16", name=f"ln16_{h}")
            nc.scalar.activation(ln16[:], l16[:], AF.Ln)
            lnp = pssc.tile([128, 1024], F32, tag="sc", name=f"lnp{h}")
            nc.tensor.transpose(lnp[0:128, 0:16], ln16[:], id16[:])
            ln_pp = small.tile([128, 16], F32, tag="lnpp", name=f"lnpp{h}")
            nc.vector.tensor_scalar_mul(ln_pp[:], lnp[0:128, 0:16], -1.0)
            lse_neg[h] = ln_pp
            rt = lpool.tile([16, 128], F32, tag="rt16", name=f"rt16_{h}")
            nc.vector.reciprocal(rt[:], l16[:])
            rt16s[h] = rt
            ctxT.append(ctx_h)
            return ctx_h

        def norm_ctx(h):
            """ctx_h *= 1/L broadcast along partitions (ones-outer per chunk)."""
            ctx_h = ctxT[h]
            rt = rt16s[h]
            rfree = lpool.tile([1, SEQ], F32, tag="rfree", name=f"rfree{h}")
            nc.sync.dma_start(rfree[:], rt[:])
            rb = [pssc.tile([128, 1024], F32, tag="sc", name=f"rb{h}_{i}")
                  for i in range(2)]
            for n in range(SEQ // 512):
                nc.tensor.matmul(
                    rb[n // 2][0:D_K, (n % 2) * 512:(n % 2 + 1) * 512],
                    ones_big[0:1, :], rfree[:, n * 512:(n + 1) * 512],
                    start=True, stop=True)
            for i in range(2):
                nc.vector.tensor_mul(ctx_h[:, i * 1024:(i + 1) * 1024],
                                     ctx_h[:, i * 1024:(i + 1) * 1024],
                                     rb[i][0:D_K, :])

        def phase_b_step(h, qt):
            """Natural scores chunk -> exp(x/8 - lnL) = normalized attn -> DMA."""
            en = workpool.tile([128, SEQ], F32, tag="en", name=f"en{h}_{qt}")
            for half in range(2):
                ps = pssc.tile([128, 1024], F32, tag="sc",
                               name=f"sB{h}_{qt}_{half}")
                for i in range(2):
                    sl = slice(half * 1024 + i * 512, half * 1024 + (i + 1) * 512)
                    nc.tensor.matmul(
                        ps[:, i * 512:(i + 1) * 512],
                        head_ap(qT_sb, h, slice(qt * 128, (qt + 1) * 128)),
                        head_ap(kT_sb, h, sl),
                        start=True, stop=True)
                nc.scalar.activation(
                    en[:, half * 1024:(half + 1) * 1024], ps[:],
                    AF.Exp, scale=float(SCALE),
                    bias=lse_neg[h][:, qt:qt + 1])
            nc.sync.dma_start(attn_out[h, qt * 128:(qt + 1) * 128, :], en[:])

        ctxps = psbig.tile([128, SEQ], F32, tag="big", name="ctxps0")
        for kt in range(KT):
            et = a_scores(0, kt)
            a_av(0, kt, et, ctxps)
        finish_a(0, ctxps)
        for h in range(HEADS_PER_CORE):
            if h + 1 < HEADS_PER_CORE:
                nxt = psbig.tile([128, SEQ], F32, tag="big", name=f"ctxps{h+1}")
                for i in range(KT):
                    # emit B's independent matmuls between A's scores and the
                    # exp-dependent AV so the PE queue never stalls on ACT.
                    et = a_scores(h + 1, i)
                    phase_b_step(h, i)
                    a_av(h + 1, i, et, nxt)
                finish_a(h + 1, nxt)
            else:
                for i in range(KT):
                    phase_b_step(h, i)
            norm_ctx(h)

        # ---- output projection (partial over this core's 256 dims) ----
        for qt in range(KT):
            pso = pssc.tile([128, 1024], F32, tag="sc", name=f"po{qt}")
            for h in range(HEADS_PER_CORE):
                for n in range(2):
                    nc.tensor.matmul(
                        pso[:, n * 512:(n + 1) * 512],
                        ctxT[h][:, qt * 128:(qt + 1) * 128],
                        wo_sb[h][:, n * 512:(n + 1) * 512],
                        start=(h == 0), stop=(h == HEADS_PER_CORE - 1))
            outs = workpool.tile([128, D_MODEL], F32, tag="outs", name=f"os{qt}")
            nc.vector.tensor_copy(outs[:], pso[:])
            nc.sync.dma_start(out_part[qt * 128:(qt + 1) * 128, :], outs[:])


_NC_CACHE = None


def get_nc():
    global _NC_CACHE
    if _NC_CACHE is None:
        _NC_CACHE = build_nc()
    return _NC_CACHE


def make_in_maps(Q, K, V, Wq, bq, Wk, bk, Wv, bv, Wo, bo):
    in_maps = []
    qkvT = {}
    for b in range(B):
        qkvT[b] = (np.ascontiguousarray(Q[b].T), np.ascontiguousarray(K[b].T),
                   np.ascontiguousarray(V[b].T))
    for c in range(N_CORES):
        b, g = divmod(c, 4)
        cs = slice(g * D_CORE, (g + 1) * D_CORE)
        qt, kt, vt = qkvT[b]
        in_maps.append({
            "qt_in": qt, "kt_in": kt, "vt_in": vt,
            "wq_in": np.ascontiguousarray(Wq[:, cs]),
            "wk_in": np.ascontiguousarray(Wk[:, cs]),
            "wv_in": np.ascontiguousarray(Wv[:, cs]),
            "wo_in": np.ascontiguousarray(Wo[cs, :]),
            "bq_in": np.ascontiguousarray(bq[cs]),
            "bk_in": np.ascontiguousarray(bk[cs]),
            "id_in": np.eye(16, dtype=np.float32),
        })
    return in_maps


def assemble(results, bv, Wo, bo):
    attn = np.empty((B, N_HEADS, SEQ, SEQ), dtype=np.float32)
    out = np.zeros((B, SEQ, D_MODEL), dtype=np.float32)
    const = (bv.astype(np.float64) @ Wo.astype(np.float64) + bo).astype(np.float32)
    for c in range(N_CORES):
        b, g = divmod(c, 4)
        attn[b, g * HEADS_PER_CORE:(g + 1) * HEADS_PER_CORE] = results[c]["attn_out"]
        out[b] += results[c]["out_part"]
    out += const
    return out, attn


def kernel(Q, K, V, Wq, bq, Wk, bk, Wv, bv, Wo, bo, _trace=False, **_ignored):
    nc = get_nc()
    in_maps = make_in_maps(Q, K, V, Wq, bq, Wk, bk, Wv, bv, Wo, bo)
    res = run_bass_kernel_spmd(nc, in_maps, list(range(N_CORES)), trace=_trace)
    out, attn = assemble(res.results, bv, Wo, bo)
    kernel.last_results = res
    return out, attn


if __name__ == "__main__":
    rng = np.random.default_rng(0)
    ins = {
        "Q": rng.standard_normal((B, SEQ, D_MODEL), dtype=np.float32),
        "K": rng.standard_normal((B, SEQ, D_MODEL), dtype=np.float32),
        "V": rng.standard_normal((B, SEQ, D_MODEL), dtype=np.float32),
    }
    s = 1.0 / np.sqrt(D_MODEL)
    for name in ("q", "k", "v", "o"):
        ins[f"W{name}"] = rng.standard_normal((D_MODEL, D_MODEL), dtype=np.float32) * s
        ins[f"b{name}"] = rng.standard_normal((D_MODEL,), dtype=np.float32) * s
    out, attn = kernel(**ins)
    print("ran", out.shape, attn.shape)
